# revision 17
# baseline (speedup 1.0000x reference)
"""Trainium2 Bass kernel for nn_ExpendMemoryUnit (scatter_memory).

Sharding: 8 cores = 4 pairs; pair p handles sample b=p. Within a pair the
full-attention j-dimension (keys/values) is split in half; the partial
unnormalized attention output + softmax denominators are summed with one
pairwise AllReduce per query block, and the tail (diff/conv64/InstanceNorm)
is software-pipelined against the next attention block. Softmax denominators
are computed by quad-accumulating exp tiles on DVE so the PE sequencer issues
4x fewer denominator matmuls; v is transposed with one DMA-xbar transpose per
chunk instead of PE transposes; conv epilogues run on ACT (single act table).
All matmuls run in bf16 with fp32 PSUM accumulation.
"""

import math
import numpy as np
import ml_dtypes
from contextlib import ExitStack

import concourse.bacc as bacc
import concourse.tile as tile
from concourse import mybir
from concourse.bass_utils import run_bass_kernel_spmd

FP32 = mybir.dt.float32
BF16 = mybir.dt.bfloat16
AF = mybir.ActivationFunctionType
ALU = mybir.AluOpType
bf = ml_dtypes.bfloat16

C = 128
S = 64
HW = S * S            # 4096
JL = HW // 2          # 2048 local keys per core
B = 4
NCORES = 8
PW = S + 2            # padded row width 66
INV1152 = 1.0 / math.sqrt(C * 9.0)

_CACHE = {}


def _taps():
    for dy in range(3):
        for dx in range(3):
            yield dy * 3 + dx, dy, dx


def _build_program(n_iters=1, skip_collective=False):
    nc = bacc.Bacc("TRN2", target_bir_lowering=False, debug=False,
                   num_devices=NCORES)

    def inp(name, shape, dtype):
        return nc.dram_tensor(name, list(shape), dtype, kind="ExternalInput").ap()

    # per-core inputs
    x_cs = inp("x_cs", [C, HW], BF16)
    x_kv = inp("x_kv", [C, 34 * S], BF16)
    kp = inp("kp", [30, 1], BF16)
    # replicated inputs (packed)
    wA = inp("wA", [C, 512], BF16)          # projT|qwT|diffT|rw1aT
    wB = inp("wB", [C, 1185], BF16)         # rw2T|modT|att1T|s2Tb
    wC = inp("wC", [C, 2304], BF16)         # scw9T|c64T
    dynT = inp("dynT", [C, 4 * 9 * 256], BF16)
    mbpad = inp("mbpad", [C, (S + 2) * PW], BF16)
    pC = inp("pC", [C, 20], FP32)
    rw1bT = inp("rw1bT", [30, C], BF16)
    att2T = inp("att2T", [33, 4], BF16)
    attb2 = inp("attb2", [1, 4], FP32)

    y = nc.dram_tensor("y", [C, HW], FP32, kind="ExternalOutput").ap()

    with tile.TileContext(nc) as tc, ExitStack() as ctx:
        sb = ctx.enter_context(tc.tile_pool(name="sb", bufs=1))
        st = ctx.enter_context(tc.tile_pool(name="st", bufs=2))
        ps = ctx.enter_context(tc.tile_pool(name="ps", bufs=1, space="PSUM"))
        dram = ctx.enter_context(tc.tile_pool(name="dram", bufs=1, space="DRAM"))

        def sbt(name, shape, dtype, **kw):
            return sb.tile(shape, dtype, name=name, **kw)

        # ---- load everything into SBUF (x_cs chunked so feat conv starts early)
        wA_sb = sb.tile_from(wA)
        pC_sb = sb.tile_from(pC)
        xcs_sb = sb.tile([C, HW], BF16, name="xcs_sb")
        for _ib in range(4):
            nc.sync.dma_start(xcs_sb[:, _ib * 1024:(_ib + 1) * 1024],
                              x_cs[:, _ib * 1024:(_ib + 1) * 1024])
        kp_sb = sb.tile_from(kp)
        rw1bT_sb = sb.tile_from(rw1bT)
        att2T_sb = sb.tile_from(att2T)
        attb2_sb = sb.tile_from(attb2)
        xkv_sb = sb.tile_from(x_kv)
        wB_sb = sb.tile_from(wB)
        mbpad_sb = sb.tile_from(mbpad)
        wC_sb = sb.tile_from(wC)
        dynT_sb = sb.tile_from(dynT)

        projT = wA_sb[:, 0:128]
        qwT = wA_sb[:, 128:256]
        diffT = wA_sb[:, 256:384]
        rw1aT = wA_sb[:, 384:512]
        rw2T = wB_sb[:, 0:512]
        modT = wB_sb[:, 512:1024]
        att1T = wB_sb[:, 1024:1057]
        s2Tb = wB_sb[:, 1057:1185]
        scw9T = wC_sb[:, 0:1152]
        c64T = wC_sb[:, 1152:2304]
        pcol = lambda i: pC_sb[:, i:i + 1]
        pb_c, rb1_c, modb_c, actb_c = pcol(0), pcol(1), pcol(2), pcol(3)
        qb_c, diffb_c, ing_c, inb_c = pcol(4), pcol(5), pcol(6), pcol(7)
        rb2_sl = pC_sb[:, 8:12]
        dynb2_sl = pC_sb[:, 12:20]

        ones_bf = sbt("ones_bf", [C, 1], BF16)
        nc.vector.memset(ones_bf[:], 1.0)
        ones_row = sbt("ones_row", [1, C], FP32)
        nc.vector.memset(ones_row[:], 1.0)
        eps8 = sbt("eps8", [C, 1], FP32)
        nc.vector.memset(eps8[:], 1e-8)
        eps5 = sbt("eps5", [C, 1], FP32)
        nc.vector.memset(eps5[:], 1e-5)

        def _emit_iter():
            # pre-zero padded scratch while engines idle
            fkv_pad = sbt("fkv_pad", [C, 34 * PW], BF16)
            nc.gpsimd.memset(fkv_pad[:], 0.0)
            c64in_pad = sbt("c64in_pad", [C, (S + 2) * PW], BF16)
            nc.gpsimd.memset(c64in_pad[:], 0.0)
            fkv3 = fkv_pad[:].rearrange("p (r c) -> p r c", c=PW)
            mp3 = mbpad_sb[:].rearrange("p (r c) -> p r c", c=PW)
            cp3 = c64in_pad[:].rearrange("p (r c) -> p r c", c=PW)
            feat3 = None  # set after feat_bf

            # ---- phase A: feat conv (1x1) from x_cs; per-channel sums for style
            feat_bf = sbt("feat_bf", [C, HW], BF16)
            fsum8 = sbt("fsum8", [C, 8], FP32)
            for cc in range(8):
                sl = slice(cc * 512, (cc + 1) * 512)
                fp = ps.tile([C, 512], FP32, name="cvp", tag="A", bufs=2)
                nc.tensor.matmul(fp[:], projT, xcs_sb[:, sl], start=True, stop=True)
                nc.scalar.activation(feat_bf[:, sl], fp[:], AF.Prelu, alpha=1.0,
                                     bias=pb_c, accum_out=fsum8[:, cc:cc + 1])
            feat3 = feat_bf[:].rearrange("p (r c) -> p r c", c=S)

            # feat on the kv halo window (34 rows), into a column-padded buffer
            for blk in range(5):
                c0 = blk * 512
                n = min(512, 34 * S - c0)
                kvp = ps.tile([C, 512], FP32, name="cvp", tag="A", bufs=2)
                nc.tensor.matmul(kvp[:, 0:n], projT, xkv_sb[:, c0:c0 + n],
                                 start=True, stop=True)
                r0 = c0 // S
                nr = n // S
                nc.scalar.activation(
                    fkv3[:, r0:r0 + nr, 1:1 + S],
                    kvp[:, 0:n].rearrange("p (r c) -> p r c", c=S),
                    AF.Prelu, alpha=1.0, bias=pb_c)

            # ---- q conv (independent of style; keeps PE busy)
            qT_sb = sbt("qT_sb", [C, HW], BF16)
            for cc in range(8):
                sl = slice(cc * 512, (cc + 1) * 512)
                qp = ps.tile([C, 512], FP32, name="cvp", tag="A", bufs=2)
                nc.tensor.matmul(qp[:], qwT, feat_bf[:, sl], start=True, stop=True)
                nc.scalar.activation(qT_sb[:, sl], qp[:], AF.Relu, bias=qb_c)

            # ---- style MLP chain (tiny)
            fsum1 = sbt("fsum1", [C, 1], FP32)
            nc.vector.reduce_sum(fsum1[:], fsum8[:], axis=mybir.AxisListType.X)
            fsum_bf = sbt("fsum_bf", [C, 1], BF16)
            nc.vector.tensor_copy(fsum_bf[:], fsum1[:])
            h1ps = ps.tile([C, 1], FP32, name="h1ps", tag="CC")
            nc.tensor.matmul(h1ps[:], rw1aT, fsum_bf[:], start=True, stop=False)
            nc.tensor.matmul(h1ps[:], rw1bT_sb[:], kp_sb[:], start=False, stop=True)
            h1_bf = sbt("h1_bf", [C, 1], BF16)
            nc.vector.tensor_scalar(h1_bf[:], h1ps[:], rb1_c, 0.0,
                                    ALU.add, ALU.max)
            scps = ps.tile([C, 4], FP32, name="scps", tag="CC")
            for c4 in range(4):
                nc.tensor.matmul(scps[:, c4:c4 + 1], rw2T[:, c4 * C:(c4 + 1) * C],
                                 h1_bf[:], start=True, stop=True)
            sc_bf = sbt("sc_bf", [C, 4], BF16)
            nc.vector.tensor_tensor(sc_bf[:], scps[:], rb2_sl, ALU.add)
            styps = ps.tile([C, 1], FP32, name="styps", tag="CC")
            for c4 in range(4):
                nc.tensor.matmul(styps[:], modT[:, c4 * C:(c4 + 1) * C],
                                 sc_bf[:, c4:c4 + 1], start=(c4 == 0), stop=(c4 == 3))
            style_f = sbt("style_f", [C, 1], FP32)
            nc.vector.tensor_scalar(style_f[:], styps[:], modb_c, None, ALU.add)
            # demod = rsqrt(sum_i style_i^2 * S2[i, o] + 1e-8)
            st2 = sbt("st2", [C, 1], BF16)
            nc.vector.tensor_tensor(st2[:], style_f[:], style_f[:], ALU.mult)
            s2ps = ps.tile([C, 1], FP32, name="s2ps", tag="CC")
            nc.tensor.matmul(s2ps[:], s2Tb, st2[:], start=True, stop=True)
            s2e = sbt("s2e", [C, 1], FP32)
            nc.vector.tensor_scalar(s2e[:], s2ps[:], 1e-8, None, ALU.add)
            s2r = sbt("s2r", [C, 1], FP32)
            nc.vector.reciprocal(s2r[:], s2e[:])
            demod = sbt("demod", [C, 1], FP32)
            nc.scalar.activation(demod[:], s2r[:], AF.Sqrt)

            # ---- modulated 3x3 conv on the memory bank; only the mean of the
            # leaky-relu output is needed (attention2d routing), so the ACT
            # epilogue discards the elementwise value and keeps accum_out.
            scw9s = sbt("scw9s", [C, 9 * C], BF16)
            nc.vector.tensor_scalar(scw9s[:], scw9T, style_f[:, 0:1],
                                    None, ALU.mult)
            lksum8 = sbt("lksum8", [C, 8], FP32)
            for cc in range(8):
                mps = ps.tile([C, 512], FP32, name="cvp", tag="A", bufs=2)
                r0 = cc * 8
                for d, dy, dx in _taps():
                    rhs = mp3[:, r0 + dy:r0 + dy + 8, dx:dx + S]
                    nc.tensor.matmul(mps[:], scw9s[:, d * C:(d + 1) * C], rhs,
                                     start=(d == 0), stop=(d == 8))
                lkscr = st.tile([C, 512], BF16, name="lkscr", bufs=2)
                nc.scalar.activation(lkscr[:], mps[:], AF.Prelu,
                                     bias=actb_c, scale=demod[:, 0:1], alpha=0.2,
                                     accum_out=lksum8[:, cc:cc + 1])

            # ---- attention2d routing -> att weights, broadcast to all partitions
            lksum1 = sbt("lksum1", [C, 1], FP32)
            nc.vector.reduce_sum(lksum1[:], lksum8[:], axis=mybir.AxisListType.X)
            a_bf = sbt("a_bf", [C, 1], BF16)
            nc.vector.tensor_copy(a_bf[:], lksum1[:])
            ahps = ps.tile([33, 1], FP32, name="ahps", tag="CC")
            nc.tensor.matmul(ahps[:], att1T, a_bf[:], start=True, stop=True)
            ah_bf = sbt("ah_bf", [33, 1], BF16)
            nc.vector.tensor_scalar(ah_bf[:], ahps[:], 0.0, 0.0, ALU.add, ALU.max)
            attps = ps.tile([1, 4], FP32, name="attps", tag="CC")
            nc.tensor.matmul(attps[:], ah_bf[:], att2T_sb[:], start=True, stop=True)
            attl = sbt("attl", [1, 4], FP32)
            nc.vector.tensor_tensor(attl[:], attps[:], attb2_sb[:], ALU.add)
            atte = sbt("atte", [1, 4], FP32)
            attsum = sbt("attsum", [1, 1], FP32)
            nc.scalar.activation(atte[:], attl[:], AF.Exp, scale=1.0 / 34.0,
                                 accum_out=attsum[:])
            attr = sbt("attr", [1, 1], FP32)
            nc.vector.reciprocal(attr[:], attsum[:])
            att_row = sbt("att_row", [1, 4], FP32)
            nc.vector.tensor_scalar(att_row[:], atte[:], attr[:, 0:1], None, ALU.mult)
            abps = ps.tile([C, 4], FP32, name="abps", tag="CC")
            nc.tensor.matmul(abps[:], ones_row[:], att_row[:], start=True, stop=True)
            att_bc = sbt("att_bc", [C, 4], FP32)
            nc.vector.tensor_copy(att_bc[:], abps[:])

            # ---- aggregate expert conv weights (DVE), in 3 tap-chunks of 768
            agg_wT = sbt("agg_wT", [C, 9 * 256], BF16)
            for g in range(3):
                sl0 = g * 768
                acc = st.tile([C, 768], FP32, name="aggacc")
                nc.vector.tensor_scalar(acc[:], dynT_sb[:, sl0:sl0 + 768],
                                        att_bc[:, 0:1], None, ALU.mult)
                for k in range(1, 4):
                    acc2 = st.tile([C, 768], FP32, name="aggacc")
                    nc.vector.scalar_tensor_tensor(
                        acc2[:], dynT_sb[:, k * 2304 + sl0:k * 2304 + sl0 + 768],
                        att_bc[:, k:k + 1], acc[:], ALU.mult, ALU.add)
                    acc = acc2
                nc.vector.tensor_copy(agg_wT[:, sl0:sl0 + 768], acc[:])
            ab = sbt("ab", [C, 2], FP32)
            abx = st.tile([C, 2], FP32, name="abx")
            nc.vector.tensor_scalar(abx[:], dynb2_sl[:, 0:2], att_bc[:, 0:1],
                                    None, ALU.mult)
            for k in range(1, 4):
                abx2 = st.tile([C, 2], FP32, name="abx")
                nc.vector.scalar_tensor_tensor(
                    abx2[:], dynb2_sl[:, 2 * k:2 * k + 2], att_bc[:, k:k + 1],
                    abx[:], ALU.mult, ALU.add)
                abx = abx2
            nc.vector.tensor_copy(ab[:], abx[:])

            # ---- kv conv (3x3, per-sample weights) on the local 32-row window.
            # v is transposed chunkwise by the DMA xbar into [key, chan] tiles.
            kT_sb = sbt("kT_sb", [C, JL], BF16)
            vT_sb = sbt("vT_sb", [C, JL], BF16)
            v3f = sbt("v3f", [C, 16 * C], BF16)
            v3 = v3f[:].rearrange("p (t c) -> p t c", c=C)
            for half, dst in ((0, kT_sb), (1, vT_sb)):
                for cc in range(4):
                    cps = ps.tile([C, 512], FP32, name="cvp", tag="A", bufs=2)
                    r0 = cc * 8
                    for d, dy, dx in _taps():
                        lhsT = agg_wT[:, d * 256 + half * C:d * 256 + (half + 1) * C]
                        rhs = fkv3[:, r0 + dy:r0 + dy + 8, dx:dx + S]
                        nc.tensor.matmul(cps[:], lhsT, rhs,
                                         start=(d == 0), stop=(d == 8))
                    sl = slice(cc * 512, (cc + 1) * 512)
                    nc.scalar.activation(dst[:, sl], cps[:], AF.Relu,
                                         bias=ab[:, half:half + 1])
                    if half == 1:
                        nc.sync.dma_start_transpose(
                            v3[:, cc * 4:(cc + 1) * 4, :], vT_sb[:, sl])

            # ---- flash-style attention over local keys (j), full queries (i),
            # with the diff/conv64 tail software-pipelined one block behind.
            stats48 = sbt("stats48", [C, 48], FP32)
            out64 = sbt("out64", [C, HW], BF16)
            # query blocks: 3x1024 then 2x512 so the last exposed collective
            # and its consume chain are half-sized
            qblocks = [(0, 1024), (1024, 1024), (2048, 1024),
                       (3072, 512), (3584, 512)]
            oouts = [None] * len(qblocks)

            def emit_attention(ib):
                q0, qw = qblocks[ib]
                o_stage = st.tile([C, qw], BF16, name=f"o_stage{qw}", bufs=2)
                d_stage = st.tile([1, qw], BF16, name=f"d_stage{qw}", bufs=2)
                for s2_ in range(qw // 512):
                    qsl = slice(q0 + s2_ * 512, q0 + s2_ * 512 + 512)
                    opsb = ps.tile([C, 512], FP32, name="opsb", tag="B", bufs=2)
                    denp = ps.tile([1, 512], FP32, name="denp", tag="CC")
                    for g in range(4):
                        qs = []
                        for j4 in range(4):
                            jt = g * 4 + j4
                            stp = ps.tile([C, 512], FP32, name="stp", tag="S",
                                          bufs=3)
                            nc.tensor.matmul(stp[:], kT_sb[:, jt * C:(jt + 1) * C],
                                             qT_sb[:, qsl], start=True, stop=True)
                            pt = st.tile([C, 512], BF16, name="pt", bufs=6)
                            nc.scalar.activation(pt[:], stp[:], AF.Exp, scale=0.25)
                            nc.tensor.matmul(opsb[:], v3f[:, jt * C:(jt + 1) * C],
                                             pt[:], start=(jt == 0), stop=(jt == 15))
                            qs.append(pt)
                        qa = st.tile([C, 512], BF16, name="qa", bufs=2)
                        nc.vector.tensor_tensor(qa[:], qs[0][:], qs[1][:], ALU.add)
                        qb2 = st.tile([C, 512], BF16, name="qb2", bufs=2)
                        nc.vector.tensor_tensor(qb2[:], qs[2][:], qs[3][:], ALU.add)
                        qsum = st.tile([C, 512], BF16, name="qsum", bufs=2)
                        nc.vector.tensor_tensor(qsum[:], qa[:], qb2[:], ALU.add)
                        nc.tensor.matmul(denp[:], ones_bf[:], qsum[:],
                                         start=(g == 0), stop=(g == 3))
                    ssl = slice(s2_ * 512, s2_ * 512 + 512)
                    nc.vector.tensor_copy(o_stage[:, ssl], opsb[:])
                    nc.vector.tensor_copy(d_stage[:, ssl], denp[:])
                oc = dram.tile([C + 1, qw], BF16, name=f"oacc{ib}")
                nc.sync.dma_start(oc[0:C, :], o_stage[:])
                nc.sync.dma_start(oc[C:C + 1, :], d_stage[:])
                oout = dram.tile([C + 1, qw], BF16, name=f"oaccout{ib}")
                if skip_collective:
                    nc.sync.dma_start(oout[:], oc[:])
                else:
                    nc.gpsimd.collective_compute(
                        "AllReduce", ALU.add,
                        replica_groups=[[0, 1], [2, 3], [4, 5], [6, 7]],
                        ins=[oc[:]], outs=[oout[:]],
                    )
                oouts[ib] = oout

            def emit_c64_chunk(cc):
                cp = ps.tile([C, 512], FP32, name="cvp", tag="A", bufs=2)
                r0 = cc * 8
                for d, dy, dx in _taps():
                    rhs = cp3[:, r0 + dy:r0 + dy + 8, dx:dx + S]
                    nc.tensor.matmul(cp[:], c64T[:, d * C:(d + 1) * C], rhs,
                                     start=(d == 0), stop=(d == 8))
                nc.vector.bn_stats(stats48[:, cc * 6:(cc + 1) * 6], cp[:])
                nc.vector.tensor_copy(out64[:, cc * 512:(cc + 1) * 512], cp[:])

            def emit_consume(ib):
                oout = oouts[ib]
                q0, qw = qblocks[ib]
                bsl = slice(q0, q0 + qw)
                o_blk = st.tile([C, qw], BF16, name=f"o_blk{qw}", bufs=2)
                nc.gpsimd.dma_start(o_blk[:], oout[0:C, :])
                den_row = st.tile([1, qw], BF16, name=f"den_row{qw}", bufs=2)
                nc.gpsimd.dma_start(den_row[:], oout[C:C + 1, :])
                recf = st.tile([1, qw], FP32, name=f"recf{qw}", bufs=2)
                nc.vector.reciprocal(recf[:], den_row[:])
                rec = st.tile([1, qw], BF16, name=f"rec{qw}", bufs=2)
                nc.vector.tensor_copy(rec[:], recf[:])
                rdb = st.tile([C, qw], BF16, name=f"rdb{qw}", bufs=2)
                nc.gpsimd.partition_broadcast(rdb[:], rec[:], channels=C)
                rn = st.tile([C, qw], BF16, name=f"rn{qw}", bufs=2)
                nc.vector.scalar_tensor_tensor(rn[:], o_blk[:], -1.0, rdb[:],
                                               ALU.mult, ALU.mult)
                dconv_in = st.tile([C, qw], BF16, name=f"dconv_in{qw}", bufs=2)
                nc.vector.tensor_tensor(dconv_in[:], rn[:], feat_bf[:, bsl],
                                        ALU.add)
                dtmp = st.tile([C, qw], BF16, name=f"dtmp{qw}", bufs=2)
                for s2_ in range(qw // 512):
                    dp = ps.tile([C, 512], FP32, name="cvp", tag="A", bufs=2)
                    nc.tensor.matmul(dp[:], diffT,
                                     dconv_in[:, s2_ * 512:s2_ * 512 + 512],
                                     start=True, stop=True)
                    nc.scalar.activation(dtmp[:, s2_ * 512:s2_ * 512 + 512],
                                         dp[:], AF.Relu, bias=diffb_c)
                r0, nr = q0 // S, qw // S
                nc.vector.tensor_tensor(
                    cp3[:, 1 + r0:1 + r0 + nr, 1:1 + S],
                    dtmp[:].rearrange("p (r c) -> p r c", c=S),
                    feat3[:, r0:r0 + nr, :], ALU.add)

            # software pipeline: attention(ib+1) overlaps consume(ib) + conv64
            c64_sched = {0: [0], 1: [1, 2], 2: [3, 4], 3: [5], 4: [6, 7]}
            emit_attention(0)
            emit_attention(1)
            emit_consume(0)
            for cc in c64_sched[0]:
                emit_c64_chunk(cc)
            emit_attention(2)
            emit_consume(1)
            for cc in c64_sched[1]:
                emit_c64_chunk(cc)
            emit_attention(3)
            emit_consume(2)
            for cc in c64_sched[2]:
                emit_c64_chunk(cc)
            emit_attention(4)
            emit_consume(3)
            for cc in c64_sched[3]:
                emit_c64_chunk(cc)
            emit_consume(4)
            for cc in c64_sched[4]:
                emit_c64_chunk(cc)

            # ---- InstanceNorm + relu epilogue
            mv = sbt("mv", [C, 2], FP32)
            nc.vector.bn_aggr(mv[:], stats48[:].rearrange("p (g k) -> p g k", k=6))
            ve = sbt("ve", [C, 1], FP32)
            nc.vector.tensor_scalar(ve[:], mv[:, 1:2], 1e-5, None, ALU.add)
            vr = sbt("vr", [C, 1], FP32)
            nc.vector.reciprocal(vr[:], ve[:])
            rsig = sbt("rsig", [C, 1], FP32)
            nc.scalar.activation(rsig[:], vr[:], AF.Sqrt)
            nsc = sbt("nsc", [C, 1], FP32)
            nc.vector.tensor_tensor(nsc[:], rsig[:], ing_c, ALU.mult)
            nt = sbt("nt", [C, 1], FP32)
            nc.vector.tensor_tensor(nt[:], mv[:, 0:1], nsc[:], ALU.mult)
            nbias = sbt("nbias", [C, 1], FP32)
            nc.vector.tensor_tensor(nbias[:], inb_c, nt[:], ALU.subtract)
            for cc in range(8):
                sl = slice(cc * 512, (cc + 1) * 512)
                ysb = st.tile([C, 512], FP32, name="ysb", bufs=3)
                if cc % 2 == 0:
                    nc.scalar.activation(ysb[:], out64[:, sl], AF.Relu,
                                         bias=nbias[:, 0:1], scale=nsc[:, 0:1])
                else:
                    yt = st.tile([C, 512], FP32, name="yt", bufs=2)
                    nc.vector.tensor_scalar(yt[:], out64[:, sl], nsc[:, 0:1],
                                            nbias[:, 0:1], ALU.mult, ALU.add)
                    nc.vector.tensor_scalar(ysb[:], yt[:], 0.0, None, ALU.max)
                nc.sync.dma_start(y[:, sl], ysb[:])

        for _it in range(n_iters):
            _emit_iter()

    nc.compile()
    return nc


def _host_prepare(inputs):
    f32 = np.float32
    feature = np.asarray(inputs["feature"], f32)
    keypoints = np.asarray(inputs["keypoints"], f32)
    mb = np.asarray(inputs["mb"], f32)

    scw = np.asarray(inputs["sc_weight"], f32)[0] * INV1152   # [C,C,3,3] o,i,dy,dx
    rep = {}
    mbp = np.zeros((C, S + 2, PW), f32)
    mbp[:, 1:1 + S, 1:1 + S] = mb[0]
    rep["mbpad"] = mbp.reshape(C, (S + 2) * PW).astype(bf)

    projT = np.asarray(inputs["proj_w"], f32)[:, :, 0, 0].T
    qwT = np.asarray(inputs["q_w"], f32)[:, :, 0, 0].T
    diffT = np.asarray(inputs["diff_w"], f32)[:, :, 0, 0].T
    rw1 = np.asarray(inputs["route_w1"], f32)                    # [128, 158]
    rw1aT = rw1[:, :C].T / HW
    rep["wA"] = np.ascontiguousarray(
        np.concatenate([projT, qwT, diffT, rw1aT], axis=1)).astype(bf)
    rep["rw1bT"] = np.ascontiguousarray(rw1[:, C:].T).astype(bf)

    rw2 = np.asarray(inputs["route_w2"], f32)                    # [512, 128]
    modw = np.asarray(inputs["mod_w"], f32) / math.sqrt(512.0)   # [128, 512]
    modT = modw.T.reshape(4, C, C).transpose(1, 0, 2).reshape(C, 4 * C)
    att1 = np.asarray(inputs["att_w1"], f32)[:, :, 0, 0]         # [33, 128]
    att1T = att1.T * (math.sqrt(2.0) / HW)
    s2T = (scw ** 2).sum(axis=(2, 3)).T                          # [i, o]
    rep["wB"] = np.ascontiguousarray(
        np.concatenate([rw2.T, modT, att1T, s2T], axis=1)).astype(bf)

    scw9T = scw.transpose(1, 2, 3, 0).reshape(C, 9 * C)          # [i,(dy dx) o]
    c64w = np.asarray(inputs["conv64_w"], f32)                   # [o,i,3,3]
    c64T = c64w.transpose(1, 2, 3, 0).reshape(C, 9 * C)
    rep["wC"] = np.ascontiguousarray(
        np.concatenate([scw9T, c64T], axis=1)).astype(bf)

    dynw = np.asarray(inputs["dyn_w"], f32)                      # [4,256,128,3,3]
    rep["dynT"] = np.ascontiguousarray(
        dynw.transpose(2, 0, 3, 4, 1).reshape(C, 4 * 9 * 256)).astype(bf)

    dynb = np.asarray(inputs["dyn_b"], f32)                      # [4, 256]
    pc = np.zeros((C, 20), f32)
    pc[:, 0] = np.asarray(inputs["proj_b"], f32)
    pc[:, 1] = np.asarray(inputs["route_b1"], f32)
    pc[:, 2] = np.asarray(inputs["mod_b"], f32)
    pc[:, 3] = np.asarray(inputs["act_b"], f32)
    pc[:, 4] = np.asarray(inputs["q_b"], f32)
    pc[:, 5] = np.asarray(inputs["diff_b"], f32)
    pc[:, 6] = np.asarray(inputs["in_g"], f32)
    pc[:, 7] = np.asarray(inputs["in_b"], f32)
    pc[:, 8:12] = np.asarray(inputs["route_b2"], f32).reshape(4, C).T
    for k in range(4):
        for blk in range(2):
            pc[:, 12 + 2 * k + blk] = dynb[k, blk * C:(blk + 1) * C]
    rep["pC"] = pc

    att2 = np.asarray(inputs["att_w2"], f32)[:, :, 0, 0]         # [4, 33]
    rep["att2T"] = np.ascontiguousarray(att2.T).astype(bf)
    rep["attb2"] = np.asarray(inputs["att_b2"], f32).reshape(1, 4)

    per_core = []
    for c in range(NCORES):
        b, h = c // 2, c % 2
        img = feature[b, 1::2]                                   # [C, 64, 64]
        d = {"x_cs": img.reshape(C, HW).astype(bf),
             "kp": keypoints[b].reshape(30, 1).astype(bf)}
        buf = np.zeros((C, 34, S), f32)
        lo = 32 * h - 1
        for j in range(34):
            r = lo + j
            if 0 <= r < S:
                buf[:, j] = img[:, r]
        d["x_kv"] = buf.reshape(C, 34 * S).astype(bf)
        per_core.append(d)
    return rep, per_core


def kernel(**inputs):
    if "nc" not in _CACHE:
        _CACHE["nc"] = _build_program()
    nc = _CACHE["nc"]
    rep, per_core = _host_prepare(inputs)
    in_maps = [{**rep, **pc} for pc in per_core]
    r = run_bass_kernel_spmd(nc, in_maps, core_ids=list(range(NCORES)))
    feature = np.asarray(inputs["feature"], np.float32)
    out = np.empty_like(feature)
    out[:, ::2] = feature[:, ::2]
    for b in range(B):
        out[b, 1::2] = r.results[2 * b]["y"].reshape(C, S, S)
    return out
